# revision 3
# baseline (speedup 1.0000x reference)
"""Trainium2 Bass kernel v2 for CrispComposition (bf16, batch-sharded).

out[b, i] = max_o( min(m[b, i], weight[i, o]) ) == min(m[b, i], max_o weight[i, o])

bf16 on device: harness tolerance is rel_err < 2e-2; bf16 input rounding gives
<= 2^-9 relative error (min/max only select values, no arithmetic).  Halves all
DMA traffic; m rows stay 1KB descriptors (>=512B, full DMA rate).

Sharding: batch axis / 8 cores; weight replicated, wmax computed locally.

Structure (latency-optimized):
  SP HWDGE:  w1 w2 m1 m2          (weight first: it heads the critical chain)
  DVE:       fused reduces, diags, then all mins (bf16 2x mode)
  PE:        bcast matmul per column half (ones^T @ diag)
  Act:       PSUM->SBUF bcast copy per column half
  stores:    per (load-chunk, column-half), alternating engines
"""

import numpy as np

import concourse.bacc as bacc
import concourse.mybir as mybir
from concourse.bass_utils import run_bass_kernel_spmd
from concourse.masks import make_identity
from concourse.tile import TileContext

B, IN, OUT = 4096, 512, 256
NCORES = 8
BS = B // NCORES  # 512 batch rows per core
P = 128
NT = IN // P  # 4 column blocks of wmax

BF16 = mybir.dt.bfloat16
F32 = mybir.dt.float32


def build_bass(
    n_w=2,  # number of weight-load DMAs
    n_load=2,  # number of m-load DMAs
    n_cols=2,  # column splits of the bcast/min pipeline (1 or 2)
    fuse_reduce=False,
    reduce_eng="vector",
    diag_eng="vector",
    copy_eng="scalar",  # psum->sbuf bcast copy engine, or "none" (mins read PSUM)
    min_eng="vector",
    store_mode="chunk",  # "chunk" | "chunk_col"
    store_engs=("sync",),
    load_eng="sync",
    w_eng="sync",
    swdge_path=False,  # gather-prep w load + scatter_add stores (pre-zeroed out)
    w_gather=None,  # override: gather-prep w load only
    sc_stores=None,  # override: scatter stores only
):
    if w_gather is None:
        w_gather = swdge_path
    if sc_stores is None:
        sc_stores = swdge_path
    nc = bacc.Bacc(num_swdge_queues=2) if (w_gather or sc_stores) else bacc.Bacc()
    m_in = nc.declare_dram_parameter("m", [BS, IN], BF16, isOutput=False)
    w_in = nc.declare_dram_parameter("weight", [IN, OUT], BF16, isOutput=False)
    out = nc.declare_dram_parameter("out", [BS, IN], BF16, isOutput=True)

    eng = lambda name: {
        "sync": nc.sync,
        "scalar": nc.scalar,
        "vector": nc.vector,
        "gpsimd": nc.gpsimd,
    }[name]

    bpd = NT // n_w  # w blocks per DMA
    rows_chunk = BS // n_load
    ns = rows_chunk // P  # row groups per load chunk
    ck = IN // n_cols  # columns per column-split

    with TileContext(nc) as tc:
        with (
            tc.tile_pool(name="consts", bufs=1) as consts,
            tc.tile_pool(name="mpool", bufs=n_load) as mpool,
            tc.tile_pool(name="opool", bufs=1) as opool,
            tc.tile_pool(name="psum", bufs=1, space="PSUM") as psum,
        ):
            # constants (Pool engine, early, off critical path)
            ones = consts.tile([P, P], BF16)
            nc.gpsimd.memset(ones, 1.0)
            identity = consts.tile([P, P], BF16)
            make_identity(nc, identity)

            # ---- weight load(s) then m loads ----
            I16 = mybir.dt.int16
            wt4 = consts.tile([P, NT, OUT], BF16)
            w_sem = None
            if w_gather:
                # Identity-index gather through the SWDGE prepare/trigger path:
                # descriptor gen runs early on Pool, the trigger fires with no
                # HWDGE hold and no DGE delay, so the transfer starts ~700ns
                # earlier and HWDGE is free for the m loads.
                # Only partitions 0-15 of the idx tile are read (j-th index =
                # idxs[j%16, j//16]) but the interpreter validates all 128,
                # hence the clamp.
                idxs_w = consts.tile([P, IN // 16], I16)
                nc.gpsimd.iota(
                    idxs_w,
                    pattern=[[16, IN // 16]],
                    base=0,
                    channel_multiplier=1,
                    allow_small_or_imprecise_dtypes=True,
                )
                nc.gpsimd.tensor_scalar_min(idxs_w, idxs_w, IN - 1)
                w_sem = nc.alloc_semaphore("w_dma_sem")
                nc.gpsimd.dma_gather(
                    wt4[:, :, :],
                    w_in[:, :],
                    idxs_w[:, :],
                    IN,
                    IN,
                    OUT,
                    prepare_only=True,
                    sem=w_sem,
                )
                nc.gpsimd.trigger_dma(count=None)
            else:
                for j in range(n_w):
                    eng(w_eng).dma_start(
                        out=wt4[:, j * bpd : (j + 1) * bpd, :],
                        in_=w_in[j * bpd * P : (j + 1) * bpd * P, :].rearrange(
                            "(t p) d -> p t d", p=P
                        ),
                    )
            m_tiles = []
            # scatter stores need tile[p, n] <-> DRAM row lo + n*128 + p
            m_pat = "(n p) d -> p n d" if sc_stores else "(p n) d -> p n d"
            m_kw = dict(p=P) if sc_stores else dict(n=ns)
            for h in range(n_load):
                lo, hi = h * rows_chunk, (h + 1) * rows_chunk
                mt = mpool.tile([P, ns, IN], BF16, tag="m")
                eng(load_eng).dma_start(
                    out=mt, in_=m_in[lo:hi].rearrange(m_pat, **m_kw)
                )
                m_tiles.append(mt)
            z_sems = []
            zeros = None
            if sc_stores:
                # pre-zero the output (scatter_add accumulates into it); these
                # transfers ride the idle DMA window behind the m loads
                zeros = consts.tile([P, ns, IN], BF16)
                nc.gpsimd.memset(zeros, 0.0)
                for h in range(n_load):
                    lo, hi = h * rows_chunk, (h + 1) * rows_chunk
                    zs = nc.alloc_semaphore(f"z{h}_sem")
                    eng(load_eng).dma_start(
                        out=out[lo:hi].rearrange(m_pat, **m_kw), in_=zeros
                    ).then_inc(zs, 16)
                    z_sems.append(zs)

            # ---- wmax reduce (fused per w-DMA) + diag ----
            wmax4 = consts.tile([P, NT], F32)
            diag4 = consts.tile([P, IN], BF16)
            if w_gather:
                # Tile tracks the gather PREP (Pool engine tick), not the DMA
                # data; gate the reduce chain on the DMA completion sem.
                eng(reduce_eng).wait_ge(w_sem, 16)
            if fuse_reduce:
                for j in range(n_w):
                    eng(reduce_eng).reduce_max(
                        out=wmax4[:, j * bpd : (j + 1) * bpd],
                        in_=wt4[:, j * bpd : (j + 1) * bpd, :],
                        axis=mybir.AxisListType.X,
                    )
                    for t in range(j * bpd, (j + 1) * bpd):
                        eng(diag_eng).tensor_scalar_mul(
                            diag4[:, t * P : (t + 1) * P],
                            identity,
                            wmax4[:, t : t + 1],
                        )
            else:
                for t in range(NT):
                    eng(reduce_eng).reduce_max(
                        out=wmax4[:, t : t + 1],
                        in_=wt4[:, t, :],
                        axis=mybir.AxisListType.X,
                    )
                    eng(diag_eng).tensor_scalar_mul(
                        diag4[:, t * P : (t + 1) * P], identity, wmax4[:, t : t + 1]
                    )

            # ---- broadcast per column half: bc[q, i] = wmax[i] ----
            # separate PSUM tiles per half: avoids a false WAR dependency
            # (matmul k+1 waiting on the copy of half k reading the same tile)
            bc_pss = [
                psum.tile([P, ck], F32, name=f"bc{k}", tag=f"bc{k}")
                for k in range(n_cols)
            ]
            bcast = None if copy_eng == "none" else consts.tile([P, IN], BF16)
            for k in range(n_cols):
                nc.tensor.matmul(
                    bc_pss[k],
                    lhsT=ones,
                    rhs=diag4[:, k * ck : (k + 1) * ck],
                    start=True,
                    stop=True,
                )
                if copy_eng == "scalar":
                    nc.scalar.copy(
                        out=bcast[:, k * ck : (k + 1) * ck], in_=bc_pss[k]
                    )
                elif copy_eng != "none":
                    eng(copy_eng).tensor_copy(
                        out=bcast[:, k * ck : (k + 1) * ck], in_=bc_pss[k]
                    )

            # ---- mins: per (column half, row group), all on DVE ----
            o_tiles = [
                opool.tile([P, ns, IN], BF16, name=f"ot{h}", tag=f"o{h}")
                for h in range(n_load)
            ]
            n_min = n_cols * NT
            for k in range(n_cols):
                bsrc = bcast[:, k * ck : (k + 1) * ck] if bcast is not None else bc_pss[k]
                for g in range(NT):
                    h, n = g // ns, g % ns
                    idx = k * NT + g
                    # optionally offload some of the last mins to Pool so the
                    # DVE chain isn't the sole tail
                    if min_eng == "mixed":
                        e = nc.gpsimd if idx == n_min - 2 else nc.vector
                    else:
                        e = eng(min_eng)
                    e.tensor_tensor(
                        out=o_tiles[h][:, n, k * ck : (k + 1) * ck],
                        in0=m_tiles[h][:, n, k * ck : (k + 1) * ck],
                        in1=bsrc,
                        op=mybir.AluOpType.min,
                    )

            # ---- stores ----
            if sc_stores:
                # scatter_add stores: preps early on Pool (separate SWDGE
                # queues so the two triggers don't serialize each other),
                # trigger after {chunk mins (auto dep), pre-zero landed}.
                sc_sems = []
                for h in range(n_load):
                    lo = h * rows_chunk
                    idxs_sc = consts.tile(
                        [P, rows_chunk // 16], I16, name=f"idxsc{h}"
                    )
                    nc.gpsimd.iota(
                        idxs_sc,
                        pattern=[[16, rows_chunk // 16]],
                        base=lo,
                        channel_multiplier=1,
                        allow_small_or_imprecise_dtypes=True,
                    )
                    nc.gpsimd.tensor_scalar_min(idxs_sc, idxs_sc, BS - 1)
                    sc_sem = nc.alloc_semaphore(f"sc{h}_sem")
                    nc.gpsimd.dma_scatter_add(
                        out[:, :],
                        o_tiles[h][:, :, :],
                        idxs_sc[:, :],
                        rows_chunk,
                        rows_chunk,
                        IN,
                        prepare_only=True,
                        sem=sc_sem,
                        queue_num=h,
                    )
                    sc_sems.append(sc_sem)
                for h in range(n_load):
                    nc.gpsimd.wait_ge(z_sems[h], 16)
                    nc.gpsimd.trigger_dma(count=None, queue_num=h)
                # the kernel must not complete before the scatter data lands
                for s in sc_sems:
                    nc.sync.wait_ge(s, 16)
                return nc
            si = 0
            if store_mode == "chunk":
                for h in range(n_load):
                    lo, hi = h * rows_chunk, (h + 1) * rows_chunk
                    eng(store_engs[si % len(store_engs)]).dma_start(
                        out=out[lo:hi].rearrange("(p n) d -> p n d", n=ns),
                        in_=o_tiles[h],
                    )
                    si += 1
            else:  # chunk_col: per (chunk, column half)
                for h in range(n_load):
                    lo, hi = h * rows_chunk, (h + 1) * rows_chunk
                    for k in range(n_cols):
                        eng(store_engs[si % len(store_engs)]).dma_start(
                            out=out[lo:hi].rearrange("(p n) d -> p n d", n=ns)[
                                :, :, k * ck : (k + 1) * ck
                            ],
                            in_=o_tiles[h][:, :, k * ck : (k + 1) * ck],
                        )
                        si += 1

    return nc


_NC_CACHE = {}


def _get_nc(**kw):
    key = tuple(sorted(kw.items()))
    if key not in _NC_CACHE:
        nc = build_bass(**kw)
        nc.finalize()
        _NC_CACHE[key] = nc
    return _NC_CACHE[key]


def run(m, weight, build_kwargs=None, **spmd_kwargs):
    bf = np.dtype(mybir.dt.np(BF16))
    m_bf = np.ascontiguousarray(m, dtype=np.float32).astype(bf)
    w_bf = np.ascontiguousarray(weight, dtype=np.float32).astype(bf)
    nc = _get_nc(**(build_kwargs or {}))
    in_maps = [
        {"m": m_bf[c * BS : (c + 1) * BS], "weight": w_bf} for c in range(NCORES)
    ]
    res = run_bass_kernel_spmd(nc, in_maps, list(range(NCORES)), **spmd_kwargs)
    full = np.concatenate(
        [np.asarray(res.results[c]["out"]) for c in range(NCORES)], axis=0
    )
    return full.astype(np.float32), res


def kernel(m, weight):
    return run(m, weight)[0]


# revision 4
# speedup vs baseline: 1.0183x; 1.0183x over previous
"""Trainium2 Bass kernel v2 for CrispComposition (bf16, batch-sharded).

out[b, i] = max_o( min(m[b, i], weight[i, o]) ) == min(m[b, i], max_o weight[i, o])

bf16 on device: harness tolerance is rel_err < 2e-2; bf16 input rounding gives
<= 2^-9 relative error (min/max only select values, no arithmetic).  Halves all
DMA traffic; m rows stay 1KB descriptors (>=512B, full DMA rate).

Sharding: batch axis / 8 cores; weight replicated, wmax computed locally.

Structure (latency-optimized):
  SP HWDGE:  w1 w2 m1 m2          (weight first: it heads the critical chain)
  DVE:       fused reduces, diags, then all mins (bf16 2x mode)
  PE:        bcast matmul per column half (ones^T @ diag)
  Act:       PSUM->SBUF bcast copy per column half
  stores:    per (load-chunk, column-half), alternating engines
"""

import numpy as np

import concourse.bacc as bacc
import concourse.mybir as mybir
from concourse.bass_utils import run_bass_kernel_spmd
from concourse.masks import make_identity
from concourse.tile import TileContext

B, IN, OUT = 4096, 512, 256
NCORES = 8
BS = B // NCORES  # 512 batch rows per core
P = 128
NT = IN // P  # 4 column blocks of wmax

BF16 = mybir.dt.bfloat16
F32 = mybir.dt.float32


def build_bass(
    n_w=2,  # number of weight-load DMAs
    n_load=2,  # number of m-load DMAs
    n_cols=2,  # column splits of the bcast/min pipeline (1 or 2)
    fuse_reduce=False,
    reduce_eng="vector",
    diag_eng="vector",
    copy_eng="scalar",  # psum->sbuf bcast copy engine, or "none" (mins read PSUM)
    min_eng="vector",
    store_mode="chunk",  # "chunk" | "chunk_col"
    store_engs=("sync",),
    load_eng="sync",
    w_eng="sync",
    swdge_path=False,  # gather-prep w load + scatter_add stores (pre-zeroed out)
    w_gather=None,  # override: gather-prep w load only
    sc_stores=None,  # override: scatter stores only
):
    if w_gather is None:
        w_gather = swdge_path
    if sc_stores is None:
        sc_stores = swdge_path
    nc = bacc.Bacc(num_swdge_queues=2) if (w_gather or sc_stores) else bacc.Bacc()
    m_in = nc.declare_dram_parameter("m", [BS, IN], BF16, isOutput=False)
    w_in = nc.declare_dram_parameter("weight", [IN, OUT], BF16, isOutput=False)
    out = nc.declare_dram_parameter("out", [BS, IN], BF16, isOutput=True)

    eng = lambda name: {
        "sync": nc.sync,
        "scalar": nc.scalar,
        "vector": nc.vector,
        "gpsimd": nc.gpsimd,
    }[name]

    bpd = NT // n_w  # w blocks per DMA
    rows_chunk = BS // n_load
    ns = rows_chunk // P  # row groups per load chunk
    ck = IN // n_cols  # columns per column-split

    with TileContext(nc) as tc:
        with (
            tc.tile_pool(name="consts", bufs=1) as consts,
            tc.tile_pool(name="mpool", bufs=n_load) as mpool,
            tc.tile_pool(name="opool", bufs=1) as opool,
            tc.tile_pool(name="psum", bufs=1, space="PSUM") as psum,
        ):
            # constants (Pool engine, early, off critical path)
            ones = consts.tile([P, P], BF16)
            nc.gpsimd.memset(ones, 1.0)
            identity = consts.tile([P, P], BF16)
            make_identity(nc, identity)

            # ---- weight load(s) then m loads ----
            I16 = mybir.dt.int16
            wt4 = consts.tile([P, NT, OUT], BF16)
            w_sem = None
            if w_gather:
                # Identity-index gather through the SWDGE prepare/trigger path:
                # descriptor gen runs early on Pool, the trigger fires with no
                # HWDGE hold and no DGE delay, so the transfer starts ~700ns
                # earlier and HWDGE is free for the m loads.
                # Only partitions 0-15 of the idx tile are read (j-th index =
                # idxs[j%16, j//16]) but the interpreter validates all 128,
                # hence the clamp.
                idxs_w = consts.tile([P, IN // 16], I16)
                nc.gpsimd.iota(
                    idxs_w,
                    pattern=[[16, IN // 16]],
                    base=0,
                    channel_multiplier=1,
                    allow_small_or_imprecise_dtypes=True,
                )
                nc.gpsimd.tensor_scalar_min(idxs_w, idxs_w, IN - 1)
                w_sem = nc.alloc_semaphore("w_dma_sem")
                nc.gpsimd.dma_gather(
                    wt4[:, :, :],
                    w_in[:, :],
                    idxs_w[:, :],
                    IN,
                    IN,
                    OUT,
                    prepare_only=True,
                    sem=w_sem,
                )
                nc.gpsimd.trigger_dma(count=None)
            else:
                for j in range(n_w):
                    eng(w_eng).dma_start(
                        out=wt4[:, j * bpd : (j + 1) * bpd, :],
                        in_=w_in[j * bpd * P : (j + 1) * bpd * P, :].rearrange(
                            "(t p) d -> p t d", p=P
                        ),
                    )
            m_tiles = []
            # scatter stores need tile[p, n] <-> DRAM row lo + n*128 + p
            m_pat = "(n p) d -> p n d" if sc_stores else "(p n) d -> p n d"
            m_kw = dict(p=P) if sc_stores else dict(n=ns)
            for h in range(n_load):
                lo, hi = h * rows_chunk, (h + 1) * rows_chunk
                mt = mpool.tile([P, ns, IN], BF16, tag="m")
                eng(load_eng).dma_start(
                    out=mt, in_=m_in[lo:hi].rearrange(m_pat, **m_kw)
                )
                m_tiles.append(mt)
            z_sems = []
            zeros = None
            if sc_stores:
                # pre-zero the output (scatter_add accumulates into it); these
                # transfers ride the idle DMA window behind the m loads
                zeros = consts.tile([P, ns, IN], BF16)
                nc.gpsimd.memset(zeros, 0.0)
                for h in range(n_load):
                    lo, hi = h * rows_chunk, (h + 1) * rows_chunk
                    zs = nc.alloc_semaphore(f"z{h}_sem")
                    eng(load_eng).dma_start(
                        out=out[lo:hi].rearrange(m_pat, **m_kw), in_=zeros
                    ).then_inc(zs, 16)
                    z_sems.append(zs)

            # ---- wmax reduce (fused per w-DMA) + diag ----
            wmax4 = consts.tile([P, NT], F32)
            diag4 = consts.tile([P, IN], BF16)
            if w_gather:
                # Tile tracks the gather PREP (Pool engine tick), not the DMA
                # data; gate the reduce chain on the DMA completion sem.
                eng(reduce_eng).wait_ge(w_sem, 16)
            if fuse_reduce:
                for j in range(n_w):
                    eng(reduce_eng).reduce_max(
                        out=wmax4[:, j * bpd : (j + 1) * bpd],
                        in_=wt4[:, j * bpd : (j + 1) * bpd, :],
                        axis=mybir.AxisListType.X,
                    )
                    for t in range(j * bpd, (j + 1) * bpd):
                        eng(diag_eng).tensor_scalar_mul(
                            diag4[:, t * P : (t + 1) * P],
                            identity,
                            wmax4[:, t : t + 1],
                        )
            else:
                for t in range(NT):
                    eng(reduce_eng).reduce_max(
                        out=wmax4[:, t : t + 1],
                        in_=wt4[:, t, :],
                        axis=mybir.AxisListType.X,
                    )
                    eng(diag_eng).tensor_scalar_mul(
                        diag4[:, t * P : (t + 1) * P], identity, wmax4[:, t : t + 1]
                    )

            # ---- broadcast per column half: bc[q, i] = wmax[i] ----
            # separate PSUM tiles per half: avoids a false WAR dependency
            # (matmul k+1 waiting on the copy of half k reading the same tile)
            bc_pss = [
                psum.tile([P, ck], F32, name=f"bc{k}", tag=f"bc{k}")
                for k in range(n_cols)
            ]
            bcast = None if copy_eng == "none" else consts.tile([P, IN], BF16)
            for k in range(n_cols):
                nc.tensor.matmul(
                    bc_pss[k],
                    lhsT=ones,
                    rhs=diag4[:, k * ck : (k + 1) * ck],
                    start=True,
                    stop=True,
                )
                if copy_eng == "scalar":
                    nc.scalar.copy(
                        out=bcast[:, k * ck : (k + 1) * ck], in_=bc_pss[k]
                    )
                elif copy_eng != "none":
                    eng(copy_eng).tensor_copy(
                        out=bcast[:, k * ck : (k + 1) * ck], in_=bc_pss[k]
                    )

            # ---- mins: per (column half, row group), all on DVE ----
            o_tiles = [
                opool.tile([P, ns, IN], BF16, name=f"ot{h}", tag=f"o{h}")
                for h in range(n_load)
            ]
            n_min = n_cols * NT
            # chunk-major order: finish all of chunk h's mins (both column
            # halves) before chunk h+1, so store h issues as early as possible
            for g in range(NT):
                h, n = g // ns, g % ns
                for k in range(n_cols):
                    bsrc = (
                        bcast[:, k * ck : (k + 1) * ck]
                        if bcast is not None
                        else bc_pss[k]
                    )
                    idx = g * n_cols + k
                    # optionally offload some of the last mins to Pool so the
                    # DVE chain isn't the sole tail
                    if min_eng == "mixed":
                        e = nc.gpsimd if idx == n_min - 2 else nc.vector
                    else:
                        e = eng(min_eng)
                    e.tensor_tensor(
                        out=o_tiles[h][:, n, k * ck : (k + 1) * ck],
                        in0=m_tiles[h][:, n, k * ck : (k + 1) * ck],
                        in1=bsrc,
                        op=mybir.AluOpType.min,
                    )

            # ---- stores ----
            if sc_stores:
                # scatter_add stores: preps early on Pool (separate SWDGE
                # queues so the two triggers don't serialize each other),
                # trigger after {chunk mins (auto dep), pre-zero landed}.
                sc_sems = []
                for h in range(n_load):
                    lo = h * rows_chunk
                    idxs_sc = consts.tile(
                        [P, rows_chunk // 16], I16, name=f"idxsc{h}"
                    )
                    nc.gpsimd.iota(
                        idxs_sc,
                        pattern=[[16, rows_chunk // 16]],
                        base=lo,
                        channel_multiplier=1,
                        allow_small_or_imprecise_dtypes=True,
                    )
                    nc.gpsimd.tensor_scalar_min(idxs_sc, idxs_sc, BS - 1)
                    sc_sem = nc.alloc_semaphore(f"sc{h}_sem")
                    nc.gpsimd.dma_scatter_add(
                        out[:, :],
                        o_tiles[h][:, :, :],
                        idxs_sc[:, :],
                        rows_chunk,
                        rows_chunk,
                        IN,
                        prepare_only=True,
                        sem=sc_sem,
                        queue_num=h,
                    )
                    sc_sems.append(sc_sem)
                for h in range(n_load):
                    nc.gpsimd.wait_ge(z_sems[h], 16)
                    nc.gpsimd.trigger_dma(count=None, queue_num=h)
                # the kernel must not complete before the scatter data lands
                for s in sc_sems:
                    nc.sync.wait_ge(s, 16)
                return nc
            si = 0
            if store_mode == "chunk":
                for h in range(n_load):
                    lo, hi = h * rows_chunk, (h + 1) * rows_chunk
                    eng(store_engs[si % len(store_engs)]).dma_start(
                        out=out[lo:hi].rearrange("(p n) d -> p n d", n=ns),
                        in_=o_tiles[h],
                    )
                    si += 1
            else:  # chunk_col: per (chunk, column half)
                for h in range(n_load):
                    lo, hi = h * rows_chunk, (h + 1) * rows_chunk
                    for k in range(n_cols):
                        eng(store_engs[si % len(store_engs)]).dma_start(
                            out=out[lo:hi].rearrange("(p n) d -> p n d", n=ns)[
                                :, :, k * ck : (k + 1) * ck
                            ],
                            in_=o_tiles[h][:, :, k * ck : (k + 1) * ck],
                        )
                        si += 1

    return nc


_NC_CACHE = {}


def _get_nc(**kw):
    key = tuple(sorted(kw.items()))
    if key not in _NC_CACHE:
        nc = build_bass(**kw)
        nc.finalize()
        _NC_CACHE[key] = nc
    return _NC_CACHE[key]


def run(m, weight, build_kwargs=None, **spmd_kwargs):
    bf = np.dtype(mybir.dt.np(BF16))
    m_bf = np.ascontiguousarray(m, dtype=np.float32).astype(bf)
    w_bf = np.ascontiguousarray(weight, dtype=np.float32).astype(bf)
    nc = _get_nc(**(build_kwargs or {}))
    in_maps = [
        {"m": m_bf[c * BS : (c + 1) * BS], "weight": w_bf} for c in range(NCORES)
    ]
    res = run_bass_kernel_spmd(nc, in_maps, list(range(NCORES)), **spmd_kwargs)
    full = np.concatenate(
        [np.asarray(res.results[c]["out"]) for c in range(NCORES)], axis=0
    )
    return full.astype(np.float32), res


def kernel(m, weight):
    return run(m, weight)[0]
